# revision 1
# baseline (speedup 1.0000x reference)
"""AffinityHead Trainium2 kernel.

Reference computation:
  f = ELU(concat(w83@conv4, w84@conv5, w85@conv6))   (1x1 convs, per pixel)
  x = ELU(w9 @ f)                                     [B, 448, 56, 56]
  aff[b,d,p] = exp(-mean_c |x[c, to(d,p)] - x[c, from(p)]|)   [B, 34, 2496]

Sharding: 8 cores = 4 images x 2 row-halves. Each core handles 26 from-rows
(+4 halo rows) = 30 rows of one image; SPMD identical program.

Device design (measured 305 us/core, rel err 7.6e-3):
- channels on partitions, pixels on free axis everywhere.
- conv inputs stream as SWDGE cast-DMAs (fp32 HBM -> bf16 SBUF, cast in
  flight): no cast pass on any engine; bf16 matmuls from 2 packed weight DMAs.
- the pair gather is pure AP arithmetic: to(d,p) = from(p) + 56*dy + dx, so
  the affinity subtract reads two shifted strided views of x (bf16). An
  odd-pixel-shifted copy of x keeps the innermost AP 4B-aligned (DVE 2x mode).
- |diff| = sign-bit strip: tensor_scalar bitwise_and 0x7fff on an int16 view
  (DVE 4x mode), every 3rd offset on ACT Abs (same act-table as Exp).
- channel reduction: ones-matmuls accumulating in PSUM, with the accumulators
  rotated across the idle conv-phase PSUM tags (pipeline depth 6);
  aff = Exp(-sum/448) on ACT straight from PSUM, row-DMA to DRAM.
- ELU = max(x, exp(min(x,0))-1): TS-min (DVE) + Exp (ACT) + fused STT (DVE).

Hard-won stack constraints (this container's walrus):
- build on bacc.Bacc and call nc.finalize(): it runs the mandatory
  one-wait-per-instruction sync legalization (generate_event_semaphores).
- AluOpType.abs_max and Pool-engine bitwise ops are not in walrus's enums.
- gpsimd ops burn a wait slot on own-engine (per-Q7 FIFO) ordering.
- keep DMAs contiguous/few: strided rearranges explode into per-line
  descriptors and overflow 16-bit semaphore wait fields.
- never read SBUF data written later in program order (no implicit barriers).

Next levers (unimplemented, need compile-verify cycles): batch ones-matmuls
into >=3.4us PE runs to escape the cold 1.2GHz HAM state; K=128 block-diag
reduction packing; fp8-DoubleRow conv6. Depth knobs are at measured optima
(dtv 12<16>20, ps rotation 2<6>8); work-shifting beyond the current split
regressed.
"""
import numpy as np
from contextlib import ExitStack

import concourse.bass as bass
from concourse import bacc
import concourse.mybir as mybir
import concourse.tile as tile
from concourse.bass_utils import run_bass_kernel_spmd

RAD = 5
W = 56
ROWS = 30            # rows of x per core (26 from + 4 halo)
FROM_ROWS = 26
NPX = ROWS * W       # 1680
NPAIR = FROM_ROWS * 48   # 1248
C = 448
N_CORES = 8

F32 = mybir.dt.float32
F32R = mybir.dt.float32r
BF16 = mybir.dt.bfloat16


def _offsets():
    out = []
    for x in range(1, RAD):
        out.append((0, x))
    for y in range(1, RAD):
        for x in range(-RAD + 1, RAD):
            if x * x + y * y < RAD * RAD:
                out.append((y, x))
    return out


OFFS = _offsets()            # 34 (dy, dx), matching reference search_dist order
assert len(OFFS) == 34

# w9 contraction split aligned to feature-group boundaries (f83|f84|f85a|f85b)
KSPLIT = [(0, 64), (64, 128), (192, 128), (320, 128)]
# x output channel groups (M-tiles of the w9 matmul)
MSPLIT = [(0, 112), (112, 112), (224, 112), (336, 112)]

SLAB = 420                   # pixel slab for PSUM-resident f/x (1 bank)
NSLAB = NPX // SLAB          # 4

# affinity chunks: (from_row0, nrows); chunk 0 only needs x rows <= 14
ACHUNKS = [(0, 11), (11, 15)]

# abs-pass engine assignment per offset index: 'v' = DVE, 'g' = GPSIMD
# three-way abs split: DVE (4x TS), GPSIMD (idle otherwise), ACT (shares
# the exp act-table). Bacc's wait legalization makes all three legal.
ABS_ENGINE = ['v'] * 34
for _i in range(2, 34, 3):
    ABS_ENGINE[_i] = 'a'   # ACT shares the exp act-table: no reloads


def _emit(ctx: ExitStack, tc: "tile.TileContext", io: dict):
    nc = tc.nc
    c6, c5, c4 = io["c6"], io["c5"], io["c4"]
    out_d = io["out"]

    persist = ctx.enter_context(tc.tile_pool(name="persist", bufs=1))
    stage6 = ctx.enter_context(tc.tile_pool(name="stage6", bufs=8))
    stage5 = ctx.enter_context(tc.tile_pool(name="stage5", bufs=2))
    stage4 = ctx.enter_context(tc.tile_pool(name="stage4", bufs=2))
    fpool = ctx.enter_context(tc.tile_pool(name="fpool", bufs=3))
    tpool = ctx.enter_context(tc.tile_pool(name="tmp", bufs=8))
    dpoolv = ctx.enter_context(tc.tile_pool(name="dtv", bufs=16))
    apool = ctx.enter_context(tc.tile_pool(name="aff", bufs=4))
    psF = ctx.enter_context(tc.tile_pool(name="psF", bufs=1, space="PSUM"))
    psX = ctx.enter_context(tc.tile_pool(name="psX", bufs=2, space="PSUM"))
    psS = ctx.enter_context(tc.tile_pool(name="psS", bufs=2, space="PSUM"))

    # ---- weights into SBUF: ONE packed f32r DMA + ONE packed bf16 DMA ----
    # (few DMAs => few semaphore waits on the barrier / first matmuls)
    wcs = persist.tile([128, 9472], BF16, name="wcs")
    nc.sync.dma_start(wcs[:], io["wc"][:])
    w9cs = persist.tile([128, 4, 448], BF16, name="w9cs")
    nc.sync.dma_start(w9cs[:], io["w9c"][:].rearrange("p (k m) -> p k m", k=4))

    def w85_sl(kt, m):
        base = kt * 256 + m * 128
        return wcs[:, base:base + 128]

    def w84_sl(kt):
        return wcs[:, 8192 + kt * 128:8192 + (kt + 1) * 128]

    def w83_sl(kt):
        return wcs[:, 9216 + kt * 64:9216 + (kt + 1) * 64]

    ones = persist.tile([112, 1], BF16, name="ones")
    nc.vector.memset(ones[:], 1.0)
    zeros = persist.tile([128, 768], BF16, name="zeros")
    nc.vector.memset(zeros[:], 0.0)
    negones = persist.tile([128, SLAB], BF16, name="negones")
    nc.vector.memset(negones[:], -1.0)
    mask16 = persist.tile([128, 768], mybir.dt.int16, name="mask16")
    nc.vector.memset(mask16[:], 32767)   # 0x7fff: clears bf16 sign bit

    # ---- x storage (bf16) + odd-shifted copy ----
    xg = [persist.tile([112, NPX], BF16, name=f"xg{g}", tag=f"xg{g}") for g in range(4)]
    xo = [persist.tile([112, NPX], BF16, name=f"xo{g}", tag=f"xo{g}") for g in range(4)]

    # ---- ELU helper: out = max(p, exp(min(p,0)) - 1), p in PSUM.
    # TensorTensor-encoded ops only (STT/TS ISA structs allow one sync wait,
    # which this flow cannot satisfy). Constants come from persistent tiles.
    def elu(psrc, dst, pn, fn):
        m = tpool.tile([pn, fn], BF16, tag="elu_m", name="elu_m")
        nc.vector.tensor_scalar(out=m[:], in0=psrc, scalar1=0.0, scalar2=None,
                                op0=mybir.AluOpType.min)
        e = tpool.tile([pn, fn], BF16, tag="elu_e", name="elu_e")
        nc.scalar.activation(out=e[:], in_=m[:],
                             func=mybir.ActivationFunctionType.Exp)
        nc.vector.scalar_tensor_tensor(
            out=dst, in0=e[:], scalar=-1.0, in1=psrc,
            op0=mybir.AluOpType.add, op1=mybir.AluOpType.max)

    # ---- conv input staging: SWDGE cast-DMA (fp32 HBM -> bf16 SBUF),
    # super-ktiles of 4x128 channels x half the pixels per transfer ----
    HALF = NPX // 2

    def stage_half(dram, n_super, h, pool):
        tiles = []
        for skt in range(n_super):
            t = pool.tile([128, 4, HALF], BF16, tag="cst", name="cst")
            view = dram[:].rearrange("(s k p) n -> s p k n", k=4, p=128)
            nc.gpsimd.dma_start(t[:], view[skt, :, :, h * HALF:(h + 1) * HALF])
            tiles.append(t)
        return tiles

    chalf = []
    for h in range(2):
        chalf.append({
            "c6": stage_half(c6, 8, h, stage6),
            "c5": stage_half(c5, 2, h, stage5),
            "c4": stage_half(c4, 1, h, stage4),
        })

    # ---- conv + x phase, slab by slab ----
    for s in range(NSLAB):
        s0 = s * SLAB
        hs = chalf[s // 2]
        j = s % 2

        def load(dram_tiles, kt):
            return dram_tiles[kt // 4][:, kt % 4, j * SLAB:(j + 1) * SLAB]

        # f85: 2 M-tiles of 128 out-ch
        f85p = [psF.tile([128, SLAB], F32, tag=f"f85{m}", name=f"f85p{m}") for m in range(2)]
        for kt in range(32):
            rhs = load(hs["c6"], kt)
            for m in range(2):
                nc.tensor.matmul(
                    f85p[m][:], w85_sl(kt, m),
                    rhs, start=(kt == 0), stop=(kt == 31))
        f84p = psF.tile([128, SLAB], F32, tag="f84", name="f84p")
        for kt in range(8):
            nc.tensor.matmul(f84p[:], w84_sl(kt), load(hs["c5"], kt),
                             start=(kt == 0), stop=(kt == 7))
        f83p = psF.tile([64, SLAB], F32, tag="f83", name="f83p")
        for kt in range(4):
            nc.tensor.matmul(f83p[:], w83_sl(kt), load(hs["c4"], kt),
                             start=(kt == 0), stop=(kt == 3))

        # ELU f -> sbuf k-group tiles (64/128/128/128 partitions)
        fk = [fpool.tile([kn, SLAB], BF16, tag=f"fk{i}", name=f"fk{i}")
              for i, (k0, kn) in enumerate(KSPLIT)]
        elu(f83p[:], fk[0][:], 64, SLAB)
        elu(f84p[:], fk[1][:], 128, SLAB)
        elu(f85p[0][:], fk[2][:], 128, SLAB)
        elu(f85p[1][:], fk[3][:], 128, SLAB)

        # x = ELU(w9 @ f): M-tiles sequential to cap PSUM use
        sl = slice(s0, s0 + SLAB)
        for mt, (m0, mn) in enumerate(MSPLIT):
            xp = psX.tile([112, SLAB], F32, tag="xp", name="xp")
            for kt in range(4):
                nc.tensor.matmul(xp[:], w9cs[0:KSPLIT[kt][1], kt, m0:m0 + mn], fk[kt][:],
                                 start=(kt == 0), stop=(kt == 3))
            elu(xp[:], xg[mt][:, sl], 112, SLAB)
        # odd-shifted copy, reading only already-written xg: window ends at
        # s0+SLAB-1 and starts one pixel back to cover the previous boundary
        start = s0 - 1 if s > 0 else 0
        for g in range(4):
            nc.vector.tensor_copy(out=xo[g][:, start:s0 + SLAB - 1],
                                  in_=xg[g][:, start + 1:s0 + SLAB])

    # ---- affinity ----
    # Rotate the per-offset PSUM accumulators across the (idle-by-now) conv
    # psum tags plus psS: 6 slots of pipeline depth between PE ones-matmuls
    # and ACT exp drains instead of 2.
    PS_SLOTS = [(psF, "f850"), (psF, "f851"), (psF, "f84"), (psF, "f83"),
                (psS, "ps"), (psS, "ps")]
    ps_k = 0
    for (r0, nr) in ACHUNKS:
        npair = nr * 48
        half = (npair // 2 + 47) // 48 * 48   # N-chunk split at 48-multiple
        nchunks = [(0, half), (half, npair - half)]
        for d_idx, (dy, dx) in enumerate(OFFS):
            dts = []
            for g in range(4):
                dt = dpoolv.tile([112, nr, 48], BF16, tag="dt", name="dtl")
                if dx % 2 == 0:
                    src, cof = xg[g], dx
                else:
                    src, cof = xo[g], dx - 1
                to_view = src[:].rearrange("p (r c) -> p r c", c=W)[
                    :, r0 + dy:r0 + dy + nr, 4 + cof:52 + cof]
                from_view = xg[g][:].rearrange("p (r c) -> p r c", c=W)[
                    :, r0:r0 + nr, 4:52]
                nc.vector.tensor_tensor(out=dt[:], in0=to_view, in1=from_view,
                                        op=mybir.AluOpType.subtract)
                dflat = dt[:].rearrange("p r c -> p (r c)")
                ae = ABS_ENGINE[d_idx]
                if ae == 'a':
                    # ACT abs (same act table as Exp: no table reload)
                    nc.scalar.activation(out=dflat, in_=dflat,
                                         func=mybir.ActivationFunctionType.Abs)
                elif ae == 'g':
                    # gpsimd lacks TensorScalarPtr; TT with the mask tile
                    di = dflat.bitcast(mybir.dt.int16)
                    nc.gpsimd.tensor_tensor(out=di, in0=di,
                                            in1=mask16[:112, :npair],
                                            op=mybir.AluOpType.bitwise_and)
                else:
                    di = dflat.bitcast(mybir.dt.int16)
                    nc.vector.tensor_scalar(out=di, in0=di, scalar1=32767,
                                            scalar2=None,
                                            op0=mybir.AluOpType.bitwise_and)
                dts.append(dflat)
            arow = apool.tile([1, npair], F32, tag="arow", name="arow")
            for (o, n) in nchunks:
                pspool, pstag = PS_SLOTS[ps_k % 6]
                ps_k += 1
                ps = pspool.tile([1, 512], F32, tag=pstag, name="ps")
                for g in range(4):
                    nc.tensor.matmul(ps[:, :n], ones[:], dts[g][:, o:o + n],
                                     start=(g == 0), stop=(g == 3))
                nc.scalar.activation(out=arow[:, o:o + n], in_=ps[:, :n],
                                     func=mybir.ActivationFunctionType.Exp,
                                     scale=-1.0 / C)
            nc.sync.dma_start(out_d[d_idx:d_idx + 1, r0 * 48:(r0 + nr) * 48],
                              arow[:])


_NC_CACHE = {}
LAST_RESULT = None


def _build_nc():
    if "nc" in _NC_CACHE:
        return _NC_CACHE["nc"]
    nc = bacc.Bacc()
    io = {
        "c6": nc.declare_dram_parameter("c6", [4096, NPX], F32, isOutput=False),
        "c5": nc.declare_dram_parameter("c5", [1024, NPX], F32, isOutput=False),
        "c4": nc.declare_dram_parameter("c4", [512, NPX], F32, isOutput=False),
        "wc": nc.declare_dram_parameter("wc", [128, 9472], BF16, isOutput=False),
        "w9c": nc.declare_dram_parameter("w9c", [128, 4 * 448], BF16, isOutput=False),
        "out": nc.declare_dram_parameter("out", [34, NPAIR], F32, isOutput=True),
    }
    with tile.TileContext(nc) as tc:
        with ExitStack() as ctx:
            _emit(ctx, tc, io)
    nc.finalize()   # Bacc.compile(): 1-wait legalization + event semaphores
    _NC_CACHE["nc"] = nc
    return nc


def _expected_indices():
    full = np.reshape(np.arange(0, 56 * 56, dtype=np.int64), (56, 56))
    ind_from = np.reshape(full[:-4, 4:-4], [-1])
    tos = []
    for dy, dx in OFFS:
        tos.append(np.reshape(full[dy:dy + 52, 4 + dx:4 + dx + 48], [-1]))
    return ind_from, np.concatenate(tos, axis=0)


def _maybe_install_trace_hook():
    import os
    if not os.environ.get("BASS_TRACE"):
        return
    import sys
    import types
    try:
        import antenv.axon_hooks  # noqa: F401
        return
    except ImportError:
        pass
    try:
        from trn_agent_boot.trn_boot import _ntff_profile_via_ctypes
        hook = _ntff_profile_via_ctypes('/opt/axon/libaxon_pjrt.so')
    except Exception:
        hook = None
    import antenv
    mod = types.ModuleType("antenv.axon_hooks")
    mod.get_axon_ntff_profile_hook = lambda: hook
    mod.set_axon_ntff_profile_hook = lambda h: None
    sys.modules["antenv.axon_hooks"] = mod
    antenv.axon_hooks = mod


def kernel(conv4, conv5, conv6, w83, w84, w85, w9, ind_from, ind_to):
    import ml_dtypes
    conv4 = np.asarray(conv4, dtype=np.float32)
    conv5 = np.asarray(conv5, dtype=np.float32)
    conv6 = np.asarray(conv6, dtype=np.float32)
    ef, et = _expected_indices()
    assert np.array_equal(np.asarray(ind_from), ef), "unexpected ind_from"
    assert np.array_equal(np.asarray(ind_to), et), "unexpected ind_to"

    def warrange(w, ktiles):
        # w [M, K] -> lhsT tiles layout [128, ktiles*M]: partition p holds
        # w.T[k*128+p, m] for each (k, m)
        wt = np.asarray(w, np.float32).T            # [K, M]
        K, M = wt.shape
        return np.ascontiguousarray(
            wt.reshape(ktiles, 128, M).transpose(1, 0, 2).reshape(128, ktiles * M))

    wc = np.ascontiguousarray(np.concatenate(
        [warrange(w85, 32), warrange(w84, 8), warrange(w83, 4)],
        axis=1)).astype(ml_dtypes.bfloat16)
    w9t_f = np.asarray(w9, np.float32).T          # [448 in, 448 out]
    w9c = np.zeros((128, 4, 448), np.float32)
    ks = [(0, 64), (64, 128), (192, 128), (320, 128)]
    for i, (k0, kn) in enumerate(ks):
        w9c[0:kn, i, :] = w9t_f[k0:k0 + kn, :]
    w9c = np.ascontiguousarray(w9c.reshape(128, 4 * 448)).astype(
        ml_dtypes.bfloat16)

    in_maps = []
    for core in range(N_CORES):
        b, half = core // 2, core % 2
        r0 = 0 if half == 0 else 26
        in_maps.append({
            "c6": np.ascontiguousarray(
                conv6[b, :, r0:r0 + ROWS, :].reshape(4096, NPX)),
            "c5": np.ascontiguousarray(
                conv5[b, :, r0:r0 + ROWS, :].reshape(1024, NPX)),
            "c4": np.ascontiguousarray(
                conv4[b, :, r0:r0 + ROWS, :].reshape(512, NPX)),
            "wc": wc, "w9c": w9c,
        })

    _maybe_install_trace_hook()
    nc = _build_nc()
    res = run_bass_kernel_spmd(nc, in_maps, list(range(N_CORES)))
    global LAST_RESULT
    LAST_RESULT = res

    aff = np.empty((4, 34, 2496), np.float32)
    for core in range(N_CORES):
        b, half = core // 2, core % 2
        aff[b, :, half * NPAIR:(half + 1) * NPAIR] = res.results[core]["out"]
    return aff



# revision 8
# speedup vs baseline: 1.0456x; 1.0456x over previous
"""AffinityHead Trainium2 kernel (v3: interleaved conv+affinity).

Reference computation:
  f = ELU(concat(w83@conv4, w84@conv5, w85@conv6))   (1x1 convs, per pixel)
  x = ELU(w9 @ f)                                     [B, 448, 56, 56]
  aff[b,d,p] = exp(-mean_c |x[c, to(d,p)] - x[c, from(p)]|)   [B, 34, 2496]

Sharding: 8 cores = 4 images x 2 row-halves. Each core handles 26 from-rows
(+4 halo rows) = 30 rows of one image; SPMD identical program.

v3 design (from v1 trace analysis: serial conv[PE/DMA ~140us] then
affinity[DVE+ACT+PE ~165us] = 305us; engines idle in the opposite phase):
- x stored as [128, 4, NPX] bf16 (448 ch padded to 4x128; pad rows zero) so
  one TT subtract + one int16-mask abs covers all channels per offset
  (fewer DVE instructions, same elements).
- affinity emitted in 3 row-bands interleaved with conv slabs
  ([s0][s1][B0 r0-9][s2][B1 r10-17][s3][B2 r18-25]) so DVE/ACT affinity work
  overlaps conv DMA/PE.
- channel reduce: ones-matmuls (K=128 incl. zero pad) with 3 offsets per
  PSUM bank at partition bases {0,32,64} (PE col-tile 32); exp batches 3
  offsets in one strided-partition ACT op (v1 did 136 exps on [1,n]).
- ones weights identical for all reduce matmuls: ldweights=False after the
  first per band (conv matmuls between bands clobber the PE array).
- sub/abs engine split across DVE (2x/4x modes), ACT Abs, gpsimd mask-TT to
  balance: DVE is the critical engine (~88us of subtracts alone).

Hard-won stack constraints (this container's walrus):
- build on bacc.Bacc and call nc.finalize() (sync legalization).
- matmul/AP base partition must be 0, 32, or 64 (96 rejected).
- keep DMAs contiguous/few; never read SBUF written later in program order.
- DVE 2x mode needs 2-byte dtype + stride-1 innermost + 4B-aligned start
  (xo = odd-shifted copy of xg serves odd-dx to_views).
"""
import numpy as np
from contextlib import ExitStack

import concourse.bass as bass
from concourse import bacc
import concourse.mybir as mybir
import concourse.tile as tile
from concourse.bass_utils import run_bass_kernel_spmd

RAD = 5
W = 56
ROWS = 30            # rows of x per core (26 from + 4 halo)
FROM_ROWS = 26
NPX = ROWS * W       # 1680
NPAIR = FROM_ROWS * 48   # 1248
C = 448
N_CORES = 8

F32 = mybir.dt.float32
BF16 = mybir.dt.bfloat16

USE_LDW_SKIP = False   # ldweights=False on repeated ones-matmuls


def _offsets():
    out = []
    for x in range(1, RAD):
        out.append((0, x))
    for y in range(1, RAD):
        for x in range(-RAD + 1, RAD):
            if x * x + y * y < RAD * RAD:
                out.append((y, x))
    return out


OFFS = _offsets()            # 34 (dy, dx), matching reference search_dist order
assert len(OFFS) == 34

# w9 contraction split aligned to feature-group boundaries (f83|f84|f85a|f85b)
KSPLIT = [(0, 64), (64, 128), (192, 128), (320, 128)]
# x output channel groups: 4 groups of <=128 (padded to 128 in storage)
MSPLIT = [(0, 128), (128, 128), (256, 128), (384, 64)]

SLAB = 420                   # pixel slab for PSUM-resident f/x (1 bank)
NSLAB = NPX // SLAB          # 4

# affinity bands: (from_row0, nrows, emit_after_slab)
# band needs x rows <= r0+nr-1+4 (+1 px for xo) => px < slab end
BANDS = [(0, 10, 1), (10, 8, 2), (18, 8, 3)]

# engine assignment per offset index: subtract and abs
# 'v' = DVE, 'a' = ACT, 'g' = GPSIMD
SUB_ENGINE = ['v'] * 34
for _i in (8, 16, 24, 32):
    SUB_ENGINE[_i] = 'g'
ABS_ENGINE = ['v'] * 34
for _i in range(1, 34, 4):
    ABS_ENGINE[_i] = 'a'   # ACT shares the exp act-table: no reloads
for _i in (6, 14, 22, 30):
    ABS_ENGINE[_i] = 'a'   # Pool engine lacks int16 bitwise ops


def _emit(ctx: ExitStack, tc: "tile.TileContext", io: dict):
    nc = tc.nc
    c6, c5, c4 = io["c6"], io["c5"], io["c4"]
    out_d = io["out"]

    persist = ctx.enter_context(tc.tile_pool(name="persist", bufs=1))
    stage6 = ctx.enter_context(tc.tile_pool(name="stage6", bufs=8))
    stage5 = ctx.enter_context(tc.tile_pool(name="stage5", bufs=2))
    stage4 = ctx.enter_context(tc.tile_pool(name="stage4", bufs=2))
    fpool = ctx.enter_context(tc.tile_pool(name="fpool", bufs=3))
    tpool = ctx.enter_context(tc.tile_pool(name="tmp", bufs=8))
    dpool = ctx.enter_context(tc.tile_pool(name="dtv", bufs=6))
    apool = ctx.enter_context(tc.tile_pool(name="aff", bufs=4))
    psF = ctx.enter_context(tc.tile_pool(name="psF", bufs=1, space="PSUM"))
    psX = ctx.enter_context(tc.tile_pool(name="psX", bufs=2, space="PSUM"))
    psA = ctx.enter_context(tc.tile_pool(name="psA", bufs=2, space="PSUM"))

    # ---- weights into SBUF: ONE packed f32r DMA + ONE packed bf16 DMA ----
    wcs = persist.tile([128, 9472], BF16, name="wcs")
    nc.sync.dma_start(wcs[:], io["wc"][:])
    w9cs = persist.tile([128, 4, 448], BF16, name="w9cs")
    nc.sync.dma_start(w9cs[:], io["w9c"][:].rearrange("p (k m) -> p k m", k=4))

    def w85_sl(kt, m):
        base = kt * 256 + m * 128
        return wcs[:, base:base + 128]

    def w84_sl(kt):
        return wcs[:, 8192 + kt * 128:8192 + (kt + 1) * 128]

    def w83_sl(kt):
        return wcs[:, 9216 + kt * 64:9216 + (kt + 1) * 64]

    ones = persist.tile([128, 1], BF16, name="ones")
    nc.vector.memset(ones[:], 1.0)
    mask16 = persist.tile([128, 1920], mybir.dt.int16, name="mask16")
    nc.vector.memset(mask16[:], 32767)   # 0x7fff: clears bf16 sign bit

    # ---- x storage (bf16, 4x128 padded groups) + odd-shifted copy ----
    xg = persist.tile([128, 4, NPX], BF16, name="xg", tag="xg")
    xo = persist.tile([128, 4, NPX], BF16, name="xo", tag="xo")
    # zero the pad rows of group 3 (channels 448..511); elu writes 0:64 only
    nc.vector.memset(xg[64:128, 3, :], 0.0)
    nc.vector.memset(xo[64:128, 3, :], 0.0)

    # ---- ELU helper: out = max(p, exp(min(p,0)) - 1), p in PSUM.
    def elu(psrc, dst, pn, fn):
        m = tpool.tile([pn, fn], BF16, tag="elu_m", name="elu_m")
        nc.vector.tensor_scalar(out=m[:], in0=psrc, scalar1=0.0, scalar2=None,
                                op0=mybir.AluOpType.min)
        e = tpool.tile([pn, fn], BF16, tag="elu_e", name="elu_e")
        nc.scalar.activation(out=e[:], in_=m[:],
                             func=mybir.ActivationFunctionType.Exp)
        nc.vector.scalar_tensor_tensor(
            out=dst, in0=e[:], scalar=-1.0, in1=psrc,
            op0=mybir.AluOpType.add, op1=mybir.AluOpType.max)

    # ---- conv input staging: SWDGE cast-DMA (fp32 HBM -> bf16 SBUF) ----
    HALF = NPX // 2

    def stage_half(dram, n_super, h, pool):
        tiles = []
        for skt in range(n_super):
            t = pool.tile([128, 4, HALF], BF16, tag="cst", name="cst")
            view = dram[:].rearrange("(s k p) n -> s p k n", k=4, p=128)
            nc.gpsimd.dma_start(t[:], view[skt, :, :, h * HALF:(h + 1) * HALF])
            tiles.append(t)
        return tiles

    chalf = []
    for h in range(2):
        chalf.append({
            "c6": stage_half(c6, 8, h, stage6),
            "c5": stage_half(c5, 2, h, stage5),
            "c4": stage_half(c4, 1, h, stage4),
        })

    xg_r = xg[:].rearrange("p g (r c) -> p g r c", c=W)
    xo_r = xo[:].rearrange("p g (r c) -> p g r c", c=W)

    def emit_band(r0, nr, band_idx):
        npair = nr * 48
        # PE col-tiles at {0,32,64} hold separate weights; conv matmuls
        # (full 128-wide) clobber all of them, so load ones once per
        # (band, col-tile) and reuse within the band.
        tile_loaded = [False, False, False]
        for t3 in range(12):
            k = min(3, 34 - t3 * 3)
            pst = psA.tile([128, 512], F32, tag="pst", name="pst")
            arow = apool.tile([32 * (k - 1) + 1, npair], F32, tag="arow",
                              name="arow")
            for j in range(k):
                d_idx = t3 * 3 + j
                dy, dx = OFFS[d_idx]
                dt = dpool.tile([128, 4, nr, 48], BF16, tag="dt", name="dt")
                if dx % 2 == 0:
                    src, cof = xg_r, dx
                else:
                    src, cof = xo_r, dx - 1
                to_view = src[:, :, r0 + dy:r0 + dy + nr, 4 + cof:52 + cof]
                from_view = xg_r[:, :, r0:r0 + nr, 4:52]
                se = SUB_ENGINE[d_idx]
                if se == 'g':
                    nc.gpsimd.tensor_tensor(out=dt[:], in0=to_view,
                                            in1=from_view,
                                            op=mybir.AluOpType.subtract)
                else:
                    nc.vector.tensor_tensor(out=dt[:], in0=to_view,
                                            in1=from_view,
                                            op=mybir.AluOpType.subtract)
                dflat = dt[:].rearrange("p g r c -> p (g r c)")
                ae = ABS_ENGINE[d_idx]
                if ae == 'a':
                    nc.scalar.activation(out=dflat, in_=dflat,
                                         func=mybir.ActivationFunctionType.Abs)
                elif ae == 'g':
                    di = dflat.bitcast(mybir.dt.int16)
                    nc.gpsimd.tensor_tensor(out=di, in0=di,
                                            in1=mask16[:, :4 * npair],
                                            op=mybir.AluOpType.bitwise_and)
                else:
                    di = dflat.bitcast(mybir.dt.int16)
                    nc.vector.tensor_scalar(out=di, in0=di, scalar1=32767,
                                            scalar2=None,
                                            op0=mybir.AluOpType.bitwise_and)
                for g in range(4):
                    mm = nc.tensor.matmul(
                        pst[32 * j:32 * j + 1, :npair], ones[:],
                        dflat[:, g * npair:(g + 1) * npair],
                        start=(g == 0), stop=(g == 3))
                    if USE_LDW_SKIP and tile_loaded[j]:
                        mm.ldweights = False
                    tile_loaded[j] = True
            # exp over partitions 0..(32k-31): covers bases {0,32,64} plus
            # don't-care rows (cost is free-size cycles; partitions parallel).
            # Strided-partition APs are rejected by the BIR verifier, so read
            # contiguously and DMA the 3 real rows individually.
            pn = 32 * (k - 1) + 1
            nc.scalar.activation(out=arow[:], in_=pst[0:pn, :npair],
                                 func=mybir.ActivationFunctionType.Exp,
                                 scale=-1.0 / C)
            for j in range(k):
                nc.sync.dma_start(
                    out_d[t3 * 3 + j:t3 * 3 + j + 1,
                          r0 * 48:(r0 + nr) * 48],
                    arow[32 * j:32 * j + 1, :])

    # ---- conv + x phase, slab by slab, bands interleaved ----
    band_q = list(BANDS)
    for s in range(NSLAB):
        s0 = s * SLAB
        hs = chalf[s // 2]
        j = s % 2

        def load(dram_tiles, kt):
            return dram_tiles[kt // 4][:, kt % 4, j * SLAB:(j + 1) * SLAB]

        # f85: 2 M-tiles of 128 out-ch
        f85p = [psF.tile([128, SLAB], F32, tag=f"f85{m}", name=f"f85p{m}") for m in range(2)]
        for kt in range(32):
            rhs = load(hs["c6"], kt)
            for m in range(2):
                nc.tensor.matmul(
                    f85p[m][:], w85_sl(kt, m),
                    rhs, start=(kt == 0), stop=(kt == 31))
        f84p = psF.tile([128, SLAB], F32, tag="f84", name="f84p")
        for kt in range(8):
            nc.tensor.matmul(f84p[:], w84_sl(kt), load(hs["c5"], kt),
                             start=(kt == 0), stop=(kt == 7))
        f83p = psF.tile([64, SLAB], F32, tag="f83", name="f83p")
        for kt in range(4):
            nc.tensor.matmul(f83p[:], w83_sl(kt), load(hs["c4"], kt),
                             start=(kt == 0), stop=(kt == 3))

        # ELU f -> sbuf k-group tiles (64/128/128/128 partitions)
        fk = [fpool.tile([kn, SLAB], BF16, tag=f"fk{i}", name=f"fk{i}")
              for i, (k0, kn) in enumerate(KSPLIT)]
        elu(f83p[:], fk[0][:], 64, SLAB)
        elu(f84p[:], fk[1][:], 128, SLAB)
        elu(f85p[0][:], fk[2][:], 128, SLAB)
        elu(f85p[1][:], fk[3][:], 128, SLAB)

        # x = ELU(w9 @ f): M-tiles sequential to cap PSUM use
        sl = slice(s0, s0 + SLAB)
        for mt, (m0, mn) in enumerate(MSPLIT):
            xp = psX.tile([mn, SLAB], F32, tag="xp", name="xp")
            for kt in range(4):
                nc.tensor.matmul(xp[:], w9cs[0:KSPLIT[kt][1], kt, m0:m0 + mn],
                                 fk[kt][:], start=(kt == 0), stop=(kt == 3))
            elu(xp[:], xg[0:mn, mt, sl], mn, SLAB)
        # odd-shifted copy, reading only already-written xg
        start = s0 - 1 if s > 0 else 0
        nc.vector.tensor_copy(out=xo[:, :, start:s0 + SLAB - 1],
                              in_=xg[:, :, start + 1:s0 + SLAB])

        # emit any affinity bands that are ready after this slab
        while band_q and band_q[0][2] == s + 1:
            r0, nr, _ = band_q.pop(0)
            emit_band(r0, nr, len(BANDS) - len(band_q) - 1)


_NC_CACHE = {}
LAST_RESULT = None


def _build_nc():
    if "nc" in _NC_CACHE:
        return _NC_CACHE["nc"]
    nc = bacc.Bacc()
    io = {
        "c6": nc.declare_dram_parameter("c6", [4096, NPX], F32, isOutput=False),
        "c5": nc.declare_dram_parameter("c5", [1024, NPX], F32, isOutput=False),
        "c4": nc.declare_dram_parameter("c4", [512, NPX], F32, isOutput=False),
        "wc": nc.declare_dram_parameter("wc", [128, 9472], BF16, isOutput=False),
        "w9c": nc.declare_dram_parameter("w9c", [128, 4 * 448], BF16, isOutput=False),
        "out": nc.declare_dram_parameter("out", [34, NPAIR], F32, isOutput=True),
    }
    with tile.TileContext(nc) as tc:
        with ExitStack() as ctx:
            _emit(ctx, tc, io)
    nc.finalize()   # Bacc.compile(): 1-wait legalization + event semaphores
    _NC_CACHE["nc"] = nc
    return nc


def _expected_indices():
    full = np.reshape(np.arange(0, 56 * 56, dtype=np.int64), (56, 56))
    ind_from = np.reshape(full[:-4, 4:-4], [-1])
    tos = []
    for dy, dx in OFFS:
        tos.append(np.reshape(full[dy:dy + 52, 4 + dx:4 + dx + 48], [-1]))
    return ind_from, np.concatenate(tos, axis=0)


def _maybe_install_trace_hook():
    import os
    if not os.environ.get("BASS_TRACE"):
        return
    import sys
    import types
    try:
        import antenv.axon_hooks  # noqa: F401
        return
    except ImportError:
        pass
    try:
        from trn_agent_boot.trn_boot import _ntff_profile_via_ctypes
        hook = _ntff_profile_via_ctypes('/opt/axon/libaxon_pjrt.so')
    except Exception:
        hook = None
    import antenv
    mod = types.ModuleType("antenv.axon_hooks")
    mod.get_axon_ntff_profile_hook = lambda: hook
    mod.set_axon_ntff_profile_hook = lambda h: None
    sys.modules["antenv.axon_hooks"] = mod
    antenv.axon_hooks = mod


def kernel(conv4, conv5, conv6, w83, w84, w85, w9, ind_from, ind_to):
    import ml_dtypes
    conv4 = np.asarray(conv4, dtype=np.float32)
    conv5 = np.asarray(conv5, dtype=np.float32)
    conv6 = np.asarray(conv6, dtype=np.float32)
    ef, et = _expected_indices()
    assert np.array_equal(np.asarray(ind_from), ef), "unexpected ind_from"
    assert np.array_equal(np.asarray(ind_to), et), "unexpected ind_to"

    def warrange(w, ktiles):
        # w [M, K] -> lhsT tiles layout [128, ktiles*M]
        wt = np.asarray(w, np.float32).T            # [K, M]
        K, M = wt.shape
        return np.ascontiguousarray(
            wt.reshape(ktiles, 128, M).transpose(1, 0, 2).reshape(128, ktiles * M))

    wc = np.ascontiguousarray(np.concatenate(
        [warrange(w85, 32), warrange(w84, 8), warrange(w83, 4)],
        axis=1)).astype(ml_dtypes.bfloat16)
    w9t_f = np.asarray(w9, np.float32).T          # [448 in, 448 out]
    w9c = np.zeros((128, 4, 448), np.float32)
    ks = [(0, 64), (64, 128), (192, 128), (320, 128)]
    for i, (k0, kn) in enumerate(ks):
        w9c[0:kn, i, :] = w9t_f[k0:k0 + kn, :]
    w9c = np.ascontiguousarray(w9c.reshape(128, 4 * 448)).astype(
        ml_dtypes.bfloat16)

    in_maps = []
    for core in range(N_CORES):
        b, half = core // 2, core % 2
        r0 = 0 if half == 0 else 26
        in_maps.append({
            "c6": np.ascontiguousarray(
                conv6[b, :, r0:r0 + ROWS, :].reshape(4096, NPX)),
            "c5": np.ascontiguousarray(
                conv5[b, :, r0:r0 + ROWS, :].reshape(1024, NPX)),
            "c4": np.ascontiguousarray(
                conv4[b, :, r0:r0 + ROWS, :].reshape(512, NPX)),
            "wc": wc, "w9c": w9c,
        })

    _maybe_install_trace_hook()
    nc = _build_nc()
    res = run_bass_kernel_spmd(nc, in_maps, list(range(N_CORES)))
    global LAST_RESULT
    LAST_RESULT = res

    aff = np.empty((4, 34, 2496), np.float32)
    for core in range(N_CORES):
        b, half = core // 2, core % 2
        aff[b, :, half * NPAIR:(half + 1) * NPAIR] = res.results[core]["out"]
    return aff
